# revision 1
# baseline (speedup 1.0000x reference)
"""Trainium2 Bass kernel for the L1 tensor-product problem.

Math (per batch row b):
  out0e = [x0e*s, CG*(x1o.v)] @ W0e * NORM0E
  out0o = [x0o*s, CG*(x1e.v)] @ W0o * NORM0O
  out1e_c = [CG*x0o*v_c, CG*x1e_c*s, CGC*cross(x1o,v)_c] @ W1e * NORM1E
  out1o_c = [CG*x0e*v_c, CG*x1o_c*s, CGC*cross(x1e,v)_c] @ W1o * NORM1O

Kernel strategy (pure data parallel over batch, 8 cores):
  * All CG/CGC/NORM constants and cross-product signs are folded into the
    weights on the host; weights are packed as 22 lhsT chunks [128K x 128M].
  * in1 is pre-transposed on the host to feature-major [1280, Bs] with the
    l=1 irreps de-interleaved to component-major rows, so the device does
    zero transposes.  in2 rows [s, v0, v1, v2] likewise as [4, Bs].
  * Every input feature is needed scaled by each of the 4 multipliers
    (s, v0, v1, v2).  Per batch tile, the 4 multipliers are broadcast
    across partitions with a K=1 ones-matmul on the PE; the 40 scaled
    [128, T] products are computed on DVE/GpSimd; 50 matmuls (float32r,
    full rate at N>=256) accumulate the 10 output chunks in PSUM;
    ScalarE copies PSUM->SBUF; DMA writes the feature-major output,
    which the host transposes back.
"""

import sys

sys.path.insert(0, "/opt/trn_rl_repo")

import numpy as np

import concourse.bass as bass
import concourse.bacc as bacc
import concourse.mybir as mybir
from concourse.bass_utils import run_bass_kernel_spmd
from concourse.tile import TileContext

N_CORES = 8
T = 512  # batch columns per tile

# irreps: 256x0e + 256x0o + 128x1e + 128x1o
CG = 1.0 / 3.0**0.5
CGC = 1.0 / 6.0**0.5
NORM0E = (1.0 / 384.0) ** 0.5
NORM0O = (1.0 / 384.0) ** 0.5
NORM1E = (3.0 / 512.0) ** 0.5
NORM1O = (3.0 / 512.0) ** 0.5

# MODE: "f32r"  - fp32 data, float32r matmuls (full-rate PE, fp32 accuracy)
#       "bf16"  - bf16 inputs/weights/products, fp32 PSUM + fp32 output
MODE = "f32r"

_BF16 = None  # lazy ml_dtypes import


def _np_dt():
    global _BF16
    if MODE == "f32r":
        return np.float32
    if _BF16 is None:
        import ml_dtypes

        _BF16 = np.dtype(ml_dtypes.bfloat16)
    return _BF16


def _dev_dt():
    # dtype of every buffer on the matmul-feeding path; the BIR verifier
    # requires producers of fp32r-matmul operands to write fp32r themselves.
    return mybir.dt.float32r if MODE == "f32r" else mybir.dt.bfloat16


def _mm_dt():
    return mybir.dt.float32r if MODE == "f32r" else mybir.dt.bfloat16


def _pack_weights(W0e, W0o, W1e, W1o):
    """Fold constants/signs; pack 22 lhsT chunks side by side: [128, 22*128]."""
    W0e = W0e.astype(np.float64) * NORM0E
    W0e[256:] *= CG
    W0o = W0o.astype(np.float64) * NORM0O
    W0o[256:] *= CG
    W1e = W1e.astype(np.float64) * NORM1E
    W1e[:384] *= CG
    W1e[384:] *= CGC
    W1o = W1o.astype(np.float64) * NORM1O
    W1o[:384] *= CG
    W1o[384:] *= CGC
    chunks = []
    for W in (W0e, W0o):  # [384, 256]
        for kc in range(3):
            for mc in range(2):
                chunks.append(W[kc * 128 : (kc + 1) * 128, mc * 128 : (mc + 1) * 128])
    for W in (W1e, W1o):  # [512, 128]
        for kc in range(4):
            chunks.append(W[kc * 128 : (kc + 1) * 128, :])
        chunks.append(-W[384:512, :])
    chunks.append(np.ones((128, 128), np.float64))  # chunk 22: ones for bcast
    packed = np.concatenate(chunks, axis=1)
    return np.ascontiguousarray(packed.astype(_np_dt()))


def _prep_shard(in1_s, in2_s):
    """in1 [Bs,1280] -> feature-major, component-deinterleaved [1280, Bs]."""
    Bs = in1_s.shape[0]
    dt = _np_dt()
    x = np.empty((1280, Bs), dt)
    x[0:512] = in1_s[:, 0:512].T
    x[512:896] = (
        in1_s[:, 512:896].reshape(Bs, 128, 3).transpose(2, 1, 0).reshape(384, Bs)
    )
    x[896:1280] = (
        in1_s[:, 896:1280].reshape(Bs, 128, 3).transpose(2, 1, 0).reshape(384, Bs)
    )
    s4 = np.ascontiguousarray(in2_s.T.astype(dt))  # rows: [s, v0, v1, v2]
    return x, s4


def _post_shard(y):
    """Device output [1280, Bs] feature-major -> [Bs, 1280] original layout."""
    Bs = y.shape[1]
    out = np.empty((Bs, 1280), np.float32)
    out[:, 0:512] = y[0:512].T
    out[:, 512:896] = y[512:896].reshape(3, 128, Bs).transpose(2, 1, 0).reshape(Bs, 384)
    out[:, 896:1280] = (
        y[896:1280].reshape(3, 128, Bs).transpose(2, 1, 0).reshape(Bs, 384)
    )
    return out


def _contribs():
    """Per output chunk oc (0..9): list of (widx, j, ch) K-contributions.

    j: 0=s, 1..3=v_c multiplier.  ch: input feature chunk 0..9
    (0,1=x0e  2,3=x0o  4+c=x1e_c  7+c=x1o_c).
    """
    C = {}
    for mc in range(2):  # out0e
        C[mc] = [(mc, 0, 0), (2 + mc, 0, 1)] + [(4 + mc, 1 + c, 7 + c) for c in range(3)]
    for mc in range(2):  # out0o
        C[2 + mc] = [(6 + mc, 0, 2), (8 + mc, 0, 3)] + [
            (10 + mc, 1 + c, 4 + c) for c in range(3)
        ]
    for c in range(3):  # out1e_c
        C[4 + c] = [
            (12, 1 + c, 2),
            (13, 1 + c, 3),
            (14, 0, 4 + c),
            (15, 1 + (c + 2) % 3, 7 + (c + 1) % 3),
            (16, 1 + (c + 1) % 3, 7 + (c + 2) % 3),
        ]
    for c in range(3):  # out1o_c
        C[7 + c] = [
            (17, 1 + c, 0),
            (18, 1 + c, 1),
            (19, 0, 7 + c),
            (20, 1 + (c + 2) % 3, 4 + (c + 1) % 3),
            (21, 1 + (c + 1) % 3, 4 + (c + 2) % 3),
        ]
    return C

# waves: out chunks processed together so each scaled product is consumed
# right after it is produced (small rotating product pool).
WAVES = [[0, 1, 2, 3], [4, 5, 6], [7, 8, 9]]


def _build_program(Bs):
    assert Bs % T == 0, (Bs, T)
    ntiles = Bs // T
    ddt = _dev_dt()
    mmdt = _mm_dt()

    nc = bacc.Bacc()
    x = nc.declare_dram_parameter("x", [1280, Bs], ddt, isOutput=False)
    s4 = nc.declare_dram_parameter("s4", [4, Bs], ddt, isOutput=False)
    w = nc.declare_dram_parameter("w", [128, 23 * 128], ddt, isOutput=False)
    y = nc.declare_dram_parameter("y", [1280, Bs], mybir.dt.float32, isOutput=True)

    contribs = _contribs()

    with TileContext(nc) as tc:
        with (
            tc.tile_pool(name="wpool", bufs=1) as wpool,
            tc.tile_pool(name="xpool", bufs=2) as xpool,
            tc.tile_pool(name="spool", bufs=2) as spool,
            tc.tile_pool(name="mbpool", bufs=2) as mbpool,
            tc.tile_pool(name="ppool", bufs=10) as ppool,
            tc.tile_pool(name="ypool", bufs=2) as ypool,
            tc.tile_pool(name="psmb", bufs=4, space="PSUM") as psmb,
            tc.tile_pool(name="pso", bufs=4, space="PSUM") as pso,
        ):
            wt = wpool.tile([128, 23 * 128], ddt)
            nc.sync.dma_start(out=wt[:, :], in_=w[:, :])
            ones = wt[0:1, 22 * 128 : 22 * 128 + 128]

            prod_k = 0  # global product counter for DVE/POOL split
            for t in range(ntiles):
                sl = slice(t * T, (t + 1) * T)
                # --- loads ---
                xt = xpool.tile([128, 10 * T], ddt, tag="xt", name="x_t")
                nc.sync.dma_start(
                    out=xt[:, :].rearrange("p (c t) -> p c t", c=10),
                    in_=x.rearrange("(c p) b -> p c b", p=128)[:, :, sl],
                )
                s4t = []
                for j in range(4):
                    sj = spool.tile([1, T], ddt, tag=f"s4{j}", name="s4_t")
                    nc.sync.dma_start(out=sj[:, :], in_=s4[j : j + 1, sl])
                    s4t.append(sj)
                mbt = []
                for j in range(4):
                    pmb = psmb.tile([128, T], mybir.dt.float32, tag="psmb", name="pmb_t")
                    nc.tensor.matmul(
                        pmb[:, :], ones, s4t[j][:, :], start=True, stop=True
                    )
                    mbj = mbpool.tile([128, T], ddt, tag=f"mb{j}", name="mb_t")
                    nc.scalar.copy(out=mbj[:, :], in_=pmb[:, :])
                    mbt.append(mbj)
                yt_full = ypool.tile(
                    [128, 10 * T], mybir.dt.float32, tag="yo", name="yt_full"
                )
                # --- waves: products + matmuls + copy-out ---
                for wave in WAVES:
                    # distinct products needed by this wave, in first-use order
                    prods = []
                    for oc in wave:
                        for (_, j, ch) in contribs[oc]:
                            if (j, ch) not in prods:
                                prods.append((j, ch))
                    ptiles = {}
                    done = {oc: 0 for oc in wave}
                    psum_t = {}
                    for (j, ch) in prods:
                        pt = ppool.tile([128, T], ddt, tag="p", name="prod_t")
                        eng = nc.gpsimd if (prod_k % 3 == 2) else nc.vector
                        eng.tensor_mul(
                            pt[:, :],
                            xt[:, ch * T : (ch + 1) * T],
                            mbt[j][:, :],
                        )
                        prod_k += 1
                        ptiles[(j, ch)] = pt
                        # emit the matmuls that consume this product
                        for oc in wave:
                            cl = contribs[oc]
                            for (widx, jj, cc) in cl:
                                if (jj, cc) != (j, ch):
                                    continue
                                if oc not in psum_t:
                                    psum_t[oc] = pso.tile(
                                        [128, T], mybir.dt.float32, tag="pso", name="pso_t"
                                    )
                                first = done[oc] == 0
                                last = done[oc] == len(cl) - 1
                                nc.tensor.matmul(
                                    psum_t[oc][:, :],
                                    wt[:, widx * 128 : (widx + 1) * 128],
                                    pt[:, :],
                                    start=first,
                                    stop=last,
                                )
                                done[oc] += 1
                                if last:
                                    nc.scalar.copy(
                                        out=yt_full[:, oc * T : (oc + 1) * T],
                                        in_=psum_t[oc][:, :],
                                    )
                    for oc in wave:
                        assert done[oc] == len(contribs[oc]), (oc, done)
                nc.sync.dma_start(
                    out=y.rearrange("(c p) b -> p c b", p=128)[:, :, sl],
                    in_=yt_full[:, :].rearrange("p (c t) -> p c t", c=10),
                )
    nc.finalize()
    return nc


_PROG_CACHE = {}


def _get_program(Bs):
    key = (Bs, MODE, T)
    if key not in _PROG_CACHE:
        _PROG_CACHE[key] = _build_program(Bs)
    return _PROG_CACHE[key]


def run(inputs, trace=False, **kw):
    in1 = np.asarray(inputs["in1"], np.float32)
    in2 = np.asarray(inputs["in2"], np.float32)
    B = in1.shape[0]
    assert B % (N_CORES * T) == 0, B
    Bs = B // N_CORES

    wpk = _pack_weights(
        np.asarray(inputs["W0e"], np.float32),
        np.asarray(inputs["W0o"], np.float32),
        np.asarray(inputs["W1e"], np.float32),
        np.asarray(inputs["W1o"], np.float32),
    )

    in_maps = []
    for i in range(N_CORES):
        ssl = slice(i * Bs, (i + 1) * Bs)
        xs, s4s = _prep_shard(in1[ssl], in2[ssl])
        in_maps.append({"x": xs, "s4": s4s, "w": wpk})

    nc = _get_program(Bs)
    res = run_bass_kernel_spmd(nc, in_maps, list(range(N_CORES)), trace=trace, **kw)

    out = np.empty((B, 1280), np.float32)
    for i in range(N_CORES):
        out[i * Bs : (i + 1) * Bs] = _post_shard(np.asarray(res.results[i]["y"]))
    return out, res


def kernel(**inputs):
    out, _ = run(inputs, trace=False)
    return out



# revision 15
# speedup vs baseline: 3.0505x; 3.0505x over previous
"""Trainium2 Bass kernel for the L1 tensor-product problem.

Math (per batch row b):
  out0e = [x0e*s, CG*(x1o.v)] @ W0e * NORM0E
  out0o = [x0o*s, CG*(x1e.v)] @ W0o * NORM0O
  out1e_c = [CG*x0o*v_c, CG*x1e_c*s, CGC*cross(x1o,v)_c] @ W1e * NORM1E
  out1o_c = [CG*x0e*v_c, CG*x1o_c*s, CGC*cross(x1e,v)_c] @ W1o * NORM1O

Kernel strategy (pure data parallel over batch, 8 cores), v2:
  * Everything bf16 on the wire and on the matmul path; PSUM accumulates
    fp32; output is written bf16 and upcast on the host (rel-err budget
    2e-2, bf16 gives ~3e-3).
  * Host packs x per core as [ntiles, 128, 10*T] so each partition's
    tile-load is one contiguous 10KB DMA descriptor (the fp32 baseline
    was descriptor-bound at ~2KB/descriptor, DMA 95% busy).
  * The per-row scalars (s, v_c) commute with the feature contraction:
    the x0o@W1e / x0e@W1o blocks are computed UNSCALED (2 matmuls each
    instead of 6 pre-scaled ones) and scaled on DVE afterwards.  This
    cuts matmuls per tile from 54 to 42.
  * The 40 pre-scaled products are emitted as 4 wide DVE instructions
    (x_all * s broadcast; x1e/x1o * v_c broadcast) using stride-0
    broadcast APs; multiplier rows are partition-broadcast by GpSimd,
    keeping the PE free of the old ones-matmul broadcasts.
"""

import sys

sys.path.insert(0, "/opt/trn_rl_repo")

import numpy as np

import concourse.bass as bass
import concourse.bacc as bacc
import concourse.mybir as mybir
from concourse.bass_utils import run_bass_kernel_spmd
from concourse.tile import TileContext

N_CORES = 8
T = 512  # batch columns per tile

# irreps: 256x0e + 256x0o + 128x1e + 128x1o
CG = 1.0 / 3.0**0.5
CGC = 1.0 / 6.0**0.5
NORM0E = (1.0 / 384.0) ** 0.5
NORM0O = (1.0 / 384.0) ** 0.5
NORM1E = (3.0 / 512.0) ** 0.5
NORM1O = (3.0 / 512.0) ** 0.5

_BF16 = None


def _bf16():
    global _BF16
    if _BF16 is None:
        import ml_dtypes

        _BF16 = np.dtype(ml_dtypes.bfloat16)
    return _BF16


def _pack_weights(W0e, W0o, W1e, W1o):
    """Fold constants/signs; 22 lhsT chunks [128,128] side by side.

    Order: 0e (kc0m0,kc0m1,kc1m0,kc1m1,kc2m0,kc2m1), 0o (same 6),
    1e (g0,g1,h,k+,k-), 1o (g0,g1,h,k+,k-).
    """
    W0e = W0e.astype(np.float64) * NORM0E
    W0e[256:] *= CG
    W0o = W0o.astype(np.float64) * NORM0O
    W0o[256:] *= CG
    W1e = W1e.astype(np.float64) * NORM1E
    W1e[:384] *= CG
    W1e[384:] *= CGC
    W1o = W1o.astype(np.float64) * NORM1O
    W1o[:384] *= CG
    W1o[384:] *= CGC
    chunks = []
    for W in (W0e, W0o):  # [384, 256]
        for kc in range(3):
            for mc in range(2):
                chunks.append(W[kc * 128 : (kc + 1) * 128, mc * 128 : (mc + 1) * 128])
    for W in (W1e, W1o):  # [512, 128]
        chunks.append(W[0:128, :])      # g0
        chunks.append(W[128:256, :])    # g1
        chunks.append(W[256:384, :])    # h
        chunks.append(W[384:512, :])    # k+
        chunks.append(-W[384:512, :])   # k-
    chunks.append(np.ones((128, 128), np.float64))  # 22: ones row for bcast
    packed = np.concatenate(chunks, axis=1)
    return np.ascontiguousarray(packed.astype(_bf16()))


def _prep_shard(in1_s, in2_s):
    """in1 [Bs,1280] -> x [nt, 128, 10*T] bf16; in2 [Bs,4] -> s4 [nt,4,T].

    Chunk order: 0,1=x0e  2,3=x0o  4+c=x1e_c  7+c=x1o_c.
    """
    Bs = in1_s.shape[0]
    nt = Bs // T
    dt = _bf16()
    x = np.empty((nt, 128, 10, T), dt)
    x[:, :, 0:4] = in1_s[:, 0:512].reshape(nt, T, 4, 128).transpose(0, 3, 2, 1)
    x[:, :, 4:7] = in1_s[:, 512:896].reshape(nt, T, 128, 3).transpose(0, 2, 3, 1)
    x[:, :, 7:10] = in1_s[:, 896:1280].reshape(nt, T, 128, 3).transpose(0, 2, 3, 1)
    s4 = np.ascontiguousarray(in2_s.reshape(nt, T, 4).transpose(0, 2, 1).astype(dt))
    return np.ascontiguousarray(x.reshape(nt, 128, 10 * T)), s4


def _post_shard(y):
    """Device y [nt, 128, 10*T] bf16 -> [Bs, 1280] fp32 original layout."""
    nt = y.shape[0]
    y = np.asarray(y).reshape(nt, 128, 10, T).astype(np.float32)
    out = np.empty((nt, T, 1280), np.float32)
    out[:, :, 0:512] = y[:, :, 0:4].transpose(0, 3, 2, 1).reshape(nt, T, 512)
    out[:, :, 512:896] = y[:, :, 4:7].transpose(0, 3, 1, 2).reshape(nt, T, 384)
    out[:, :, 896:1280] = y[:, :, 7:10].transpose(0, 3, 1, 2).reshape(nt, T, 384)
    return out.reshape(nt * T, 1280)


def _build_program(Bs):
    assert Bs % T == 0, (Bs, T)
    nt = Bs // T
    bf = mybir.dt.bfloat16
    f32 = mybir.dt.float32

    nc = bacc.Bacc()
    x = nc.declare_dram_parameter("x", [nt, 128, 10 * T], bf, isOutput=False)
    s4 = nc.declare_dram_parameter("s4", [nt, 4, T], bf, isOutput=False)
    w = nc.declare_dram_parameter("w", [128, 23 * 128], bf, isOutput=False)
    y = nc.declare_dram_parameter("y", [nt, 128, 10 * T], bf, isOutput=True)

    with TileContext(nc) as tc:
        with (
            tc.tile_pool(name="wpool", bufs=1) as wpool,
            tc.tile_pool(name="xpool", bufs=2) as xpool,
            tc.tile_pool(name="spool", bufs=2) as spool,
            tc.tile_pool(name="mbpool", bufs=2) as mbpool,
            tc.tile_pool(name="pspool", bufs=2) as pspool,
            tc.tile_pool(name="pvpool", bufs=6) as pvpool,
            tc.tile_pool(name="cpool", bufs=2) as cpool,
            tc.tile_pool(name="ypool", bufs=2) as ypool,
            tc.tile_pool(name="psum", bufs=8, space="PSUM") as psum,
        ):
            wt = wpool.tile([128, 23 * 128], bf)
            nc.sync.dma_start(out=wt[:, :], in_=w[:, :])
            ones = wt[0:1, 22 * 128 : 22 * 128 + 128]

            def W(i):
                return wt[:, i * 128 : (i + 1) * 128]

            for t in range(nt):
                # ---- loads ----
                xt = xpool.tile([128, 10 * T], bf, tag="xt", name="x_t")
                nc.sync.dma_start(out=xt[:, :], in_=x[t, :, :])
                # ---- multiplier broadcast [128, 4T] via ones-matmul ----
                mbt = mbpool.tile([128, 4 * T], bf, tag="mb", name="mb_t")
                for j in range(4):
                    s4t = spool.tile([1, T], bf, tag=f"s4{j}", name="s4_t")
                    nc.sync.dma_start(out=s4t[:, :], in_=s4[t, j : j + 1, :])
                    pmb = psum.tile([128, T], f32, tag="psmb", name="pmb_t", bufs=2)
                    nc.tensor.matmul(pmb[:, :], ones, s4t[:, :], start=True, stop=True)
                    nc.scalar.copy(out=mbt[:, j * T : (j + 1) * T], in_=pmb[:, :])

                def mb(j, nch):
                    # [128, nch, T] stride-0 broadcast of multiplier row j
                    return (
                        mbt[:, j * T : (j + 1) * T]
                        .unsqueeze(1)
                        .broadcast_to([128, nch, T])
                    )

                # ---- products: 4 wide DVE ops ----
                ps = pspool.tile([128, 10 * T], bf, tag="ps", name="ps_t")
                nc.vector.tensor_mul(
                    ps[:, :].rearrange("p (c t) -> p c t", c=10),
                    xt[:, :].rearrange("p (c t) -> p c t", c=10),
                    mb(0, 10),
                )
                pv = []
                for c in range(3):
                    pvc = pvpool.tile([128, 6 * T], bf, tag=f"pv{c}", name="pv_t")
                    nc.vector.tensor_mul(
                        pvc[:, :].rearrange("p (c t) -> p c t", c=6),
                        xt[:, 4 * T :].rearrange("p (c t) -> p c t", c=6),
                        mb(1 + c, 6),
                    )
                    pv.append(pvc)

                def PS(ch):  # s-scaled chunk
                    return ps[:, ch * T : (ch + 1) * T]

                def PV(c, ch):  # v_c-scaled chunk (ch is global 4..9)
                    return pv[c][:, (ch - 4) * T : (ch - 3) * T]

                def XT(ch):  # raw chunk
                    return xt[:, ch * T : (ch + 1) * T]

                yt = ypool.tile([128, 10 * T], bf, tag="yo", name="y_t")

                def mm_accum(contribs, name):
                    p = psum.tile([128, T], f32, tag="ps", name=name, bufs=6)
                    n = len(contribs)
                    for i, (wi, rhs) in enumerate(contribs):
                        nc.tensor.matmul(
                            p[:, :], W(wi), rhs, start=(i == 0), stop=(i == n - 1)
                        )
                    return p

                # ---- 0e / 0o : pre-scaled, 10 matmuls each ----
                for m in range(2):
                    p = mm_accum(
                        [
                            (0 * 2 + m, PS(0)),
                            (1 * 2 + m, PS(1)),
                            (2 * 2 + m, PV(0, 7)),
                            (2 * 2 + m, PV(1, 8)),
                            (2 * 2 + m, PV(2, 9)),
                        ],
                        "ps0e",
                    )
                    nc.scalar.copy(out=yt[:, m * T : (m + 1) * T], in_=p[:, :])
                for m in range(2):
                    p = mm_accum(
                        [
                            (6 + 0 * 2 + m, PS(2)),
                            (6 + 1 * 2 + m, PS(3)),
                            (6 + 2 * 2 + m, PV(0, 4)),
                            (6 + 2 * 2 + m, PV(1, 5)),
                            (6 + 2 * 2 + m, PV(2, 6)),
                        ],
                        "ps0o",
                    )
                    nc.scalar.copy(out=yt[:, (2 + m) * T : (3 + m) * T], in_=p[:, :])

                # ---- 1e / 1o : g post-scaled ----
                # (wb, xg0, hb, cb, ob): weight base, g-input chunk, h-chunk
                # base (same parity as output), cross-chunk base (opposite
                # l=1 parity), output chunk base.
                for wb, xg0, hb, cb, ob in ((12, 2, 4, 7, 4), (17, 0, 7, 4, 7)):
                    # g = x0?' @ Wg  (unscaled)
                    gp = mm_accum([(wb + 0, XT(xg0)), (wb + 1, XT(xg0 + 1))], "psg")
                    sg = cpool.tile([128, T], bf, tag="sg", name="sg_t", bufs=4)
                    nc.scalar.copy(out=sg[:, :], in_=gp[:, :])
                    for c in range(3):
                        a, b = (c + 1) % 3, (c + 2) % 3
                        p = mm_accum(
                            [
                                (wb + 2, PS(hb + c)),         # h: x1par_c * s
                                (wb + 3, PV(b, cb + a)),      # k+: x1op_a * v_b
                                (wb + 4, PV(a, cb + b)),      # k-: x1op_b * v_a
                            ],
                            "ps1",
                        )
                        sc = cpool.tile([128, T], bf, tag="sc", name="sc_t", bufs=8)
                        nc.scalar.copy(out=sc[:, :], in_=p[:, :])
                        ys = yt[:, (ob + c) * T : (ob + c + 1) * T]
                        nc.vector.tensor_mul(
                            ys, mbt[:, (1 + c) * T : (2 + c) * T], sg[:, :]
                        )
                        nc.vector.tensor_add(ys, ys, sc[:, :])

                nc.sync.dma_start(out=y[t, :, :], in_=yt[:, :])
    nc.finalize()
    return nc


_PROG_CACHE = {}


def _get_program(Bs):
    if Bs not in _PROG_CACHE:
        _PROG_CACHE[Bs] = _build_program(Bs)
    return _PROG_CACHE[Bs]


def run(inputs, trace=False, **kw):
    in1 = np.asarray(inputs["in1"], np.float32)
    in2 = np.asarray(inputs["in2"], np.float32)
    B = in1.shape[0]
    assert B % (N_CORES * T) == 0, B
    Bs = B // N_CORES

    wpk = _pack_weights(
        np.asarray(inputs["W0e"], np.float32),
        np.asarray(inputs["W0o"], np.float32),
        np.asarray(inputs["W1e"], np.float32),
        np.asarray(inputs["W1o"], np.float32),
    )

    in_maps = []
    for i in range(N_CORES):
        ssl = slice(i * Bs, (i + 1) * Bs)
        xs, s4s = _prep_shard(in1[ssl], in2[ssl])
        in_maps.append({"x": xs, "s4": s4s, "w": wpk})

    nc = _get_program(Bs)
    res = run_bass_kernel_spmd(nc, in_maps, list(range(N_CORES)), trace=trace, **kw)

    out = np.empty((B, 1280), np.float32)
    for i in range(N_CORES):
        out[i * Bs : (i + 1) * Bs] = _post_shard(res.results[i]["y"])
    return out, res


def kernel(**inputs):
    out, _ = run(inputs, trace=False)
    return out
